# revision 3
# baseline (speedup 1.0000x reference)
"""Multi-head attention (B=2, S=2048, D=1024, H=16) on 8 Trainium2 NeuronCores.

Sharding: head-parallel attention (2 heads/core for both batches), then an
8-core AllToAll redistributes the per-head attention outputs so each core
computes the output projection for one (batch, 512-row) chunk of the output.

Everything runs in float32r (TF32-like) matmuls: full fp32 storage,
~2^-12 relative rounding, full PE speed for moving dim >= 256.

Per-core layout (core c, heads hA=2c, hB=2c+1):
  - Q^T/K^T: [128(2 heads x 64), 4096(b*2048+t)] computed via lhsT=W slice,
    rhs=X^T; bias added per-partition by DVE.
  - V: computed transposed (V^T) the same way, then PE-transposed into
    V_aug tiles [128 keys, 2 heads, 65] whose last column stays 1.0 so the
    attention-value matmul also accumulates the softmax denominators.
  - scores^T (keys on partitions): two K=64 matmuls packed in 64x128 row
    tiles (head A rows 0-63, head B rows 64-127 via tile_position=(64,0)).
  - exp with fused 1/sqrt(64) scale: ScalarE, PSUM->SBUF, [128,1024] tiles.
  - AV: out^T[d(+den), i] accumulated over 16 key blocks, M=65.
  - normalize: DVE reciprocal of den row, GPSIMD partition-broadcast,
    DVE multiply -> A^T contribution [128, 4096].
  - AllToAll (8 shards of [128, 512]) -> each core holds A^T_full
    [1024, 512] for its output chunk; Wo projection + bias -> out.
"""

import numpy as np

B, S, D, H, HD = 2, 2048, 1024, 16, 64
NCORES = 8
BT = B * S  # 4096 global token index: b*2048 + t
SCALE = 1.0 / 8.0  # 1/sqrt(HD)

_CACHE = {}


def _build():
    import concourse.bacc as bacc
    import concourse.tile as tile
    import concourse.mybir as mybir

    F32R = mybir.dt.float32r
    F32 = mybir.dt.float32
    EXP = mybir.ActivationFunctionType.Exp

    nc = bacc.Bacc("TRN2", target_bir_lowering=False, debug=False,
                   num_devices=NCORES)

    # ---- I/O ------------------------------------------------------------
    xT = nc.dram_tensor("xT", [D, BT], F32R, kind="ExternalInput")
    wq = nc.dram_tensor("wq", [D, 128], F32R, kind="ExternalInput")
    wk = nc.dram_tensor("wk", [D, 128], F32R, kind="ExternalInput")
    wv = nc.dram_tensor("wv", [D, 128], F32R, kind="ExternalInput")
    bq = nc.dram_tensor("bq", [128, 1], F32, kind="ExternalInput")
    bk = nc.dram_tensor("bk", [128, 1], F32, kind="ExternalInput")
    bv = nc.dram_tensor("bv", [128, 1], F32, kind="ExternalInput")
    wo = nc.dram_tensor("wo", [D, D], F32R, kind="ExternalInput")
    bo = nc.dram_tensor("bo", [1, D], F32, kind="ExternalInput")
    eye = nc.dram_tensor("eye", [128, 128], F32R, kind="ExternalInput")
    vones = nc.dram_tensor("vones", [128, 130], F32R, kind="ExternalInput")
    out = nc.dram_tensor("out", [512, D], F32, kind="ExternalOutput")

    NQ = 4            # xT quarters (1024 tokens each)
    KB = D // 128     # 8 contraction blocks
    groups = [list(range(NCORES))]

    with tile.TileContext(nc) as tc:
        from contextlib import ExitStack
        with ExitStack() as ctx:
            persist = ctx.enter_context(tc.tile_pool(name="persist", bufs=1))
            dram = ctx.enter_context(tc.tile_pool(name="dram", bufs=1, space="DRAM"))

            # ---- constant / weight loads --------------------------------
            wqkv_sb = []
            for k in range(KB):
                t = persist.tile([128, 384], F32R, tag=f"wqkv{k}", name=f"wqkv{k}")
                nc.sync.dma_start(t[:, 0:128], wq[k * 128:(k + 1) * 128, :])
                nc.sync.dma_start(t[:, 128:256], wk[k * 128:(k + 1) * 128, :])
                nc.sync.dma_start(t[:, 256:384], wv[k * 128:(k + 1) * 128, :])
                wqkv_sb.append(t)
            bq_sb = persist.tile([128, 1], F32, tag="bq")
            bk_sb = persist.tile([128, 1], F32, tag="bk")
            bv_sb = persist.tile([128, 1], F32, tag="bv")
            nc.sync.dma_start(bq_sb[:], bq[:])
            nc.sync.dma_start(bk_sb[:], bk[:])
            nc.sync.dma_start(bv_sb[:], bv[:])
            eye_sb = persist.tile([128, 128], F32R, tag="eye")
            nc.sync.dma_start(eye_sb[:], eye[:])
            bo_row = persist.tile([1, D], F32, tag="bo_row")
            nc.sync.dma_start(bo_row[:], bo[:])
            bo_bc = persist.tile([128, D], F32, tag="bo_bc")
            nc.gpsimd.partition_broadcast(bo_bc[:], bo_row[:])

            # persistent activations
            qT = [persist.tile([128, S], F32R, tag=f"qT{b}", name=f"qT{b}") for b in range(B)]
            kT = [persist.tile([128, S], F32R, tag=f"kT{b}", name=f"kT{b}") for b in range(B)]
            aT = [persist.tile([128, S], F32R, tag=f"aT{b}", name=f"aT{b}") for b in range(B)]
            v_aug = [persist.tile([128, 2, 65], F32R, tag=f"vaug{tb}",
                                    name=f"vaug{tb}")
                     for tb in range(BT // 128)]
            for tb in range(BT // 128):
                nc.sync.dma_start(v_aug[tb][:].rearrange("p h d -> p (h d)"),
                                  vones[:])

            # wo rhs tiles (loaded early, used at the end)
            wo_sb = []
            for k in range(KB):
                t = persist.tile([128, D], F32R, tag=f"wo{k}", name=f"wo{k}")
                nc.sync.dma_start(t[:], wo[k * 128:(k + 1) * 128, :])
                wo_sb.append(t)

            # ---- projections --------------------------------------------
            with tc.tile_pool(name="xq", bufs=2 * KB + 2) as xq_pool, \
                 tc.tile_pool(name="vtmp", bufs=2) as vtmp_pool, \
                 tc.tile_pool(name="proj_ps", bufs=3, space="PSUM") as pps, \
                 tc.tile_pool(name="vt_ps", bufs=2, space="PSUM") as vps:
                for q in range(NQ):
                    b = q // 2
                    xk = []
                    for k in range(KB):
                        t = xq_pool.tile([128, 1024], F32R, tag="xq", name=f"xq{q}_{k}")
                        nc.sync.dma_start(
                            t[:], xT[k * 128:(k + 1) * 128,
                                     q * 1024:(q + 1) * 1024])
                        xk.append(t)
                    lo = (q % 2) * 1024  # within-batch column offset
                    for ch in range(2):  # 512-token chunks in this quarter
                        cs, ce = ch * 512, (ch + 1) * 512
                        # Q^T and K^T
                        for (wcol, bias, dst) in ((0, bq_sb, qT[b]),
                                                  (128, bk_sb, kT[b])):
                            acc = pps.tile([128, 512], F32, tag="pacc")
                            for k in range(KB):
                                nc.tensor.matmul(
                                    acc[:],
                                    wqkv_sb[k][:, wcol:wcol + 128],
                                    xk[k][:, cs:ce],
                                    start=(k == 0), stop=(k == KB - 1))
                            nc.vector.tensor_scalar_add(
                                dst[:, lo + cs:lo + ce], acc[:], bias[:])
                        # V^T chunk -> transpose into v_aug
                        acc = pps.tile([128, 512], F32, tag="pacc")
                        for k in range(KB):
                            nc.tensor.matmul(
                                acc[:], wqkv_sb[k][:, 256:384],
                                xk[k][:, cs:ce],
                                start=(k == 0), stop=(k == KB - 1))
                        vt = vtmp_pool.tile([128, 512], F32R, tag="vt")
                        nc.vector.tensor_scalar_add(vt[:], acc[:], bv_sb[:])
                        for blk in range(4):
                            tb = q * 8 + ch * 4 + blk
                            pv = vps.tile([128, 128], F32R, tag="pv")
                            nc.tensor.transpose(
                                pv[:], vt[:, blk * 128:(blk + 1) * 128],
                                eye_sb[:])
                            nc.vector.tensor_copy(
                                v_aug[tb][:, 0:2, 0:64],
                                pv[:].rearrange("p (h d) -> p h d", h=2))

            # ---- attention ----------------------------------------------
            with tc.tile_pool(name="pt", bufs=2) as pt_pool, \
                 tc.tile_pool(name="rc", bufs=2) as rc_pool, \
                 tc.tile_pool(name="sc_ps", bufs=1, space="PSUM") as sc_ps, \
                 tc.tile_pool(name="av_ps", bufs=1, space="PSUM") as av_ps:
                for b in range(B):
                    for ich in range(2):  # 1024-query chunks
                        qlo = ich * 1024
                        avA = av_ps.tile([65, 1024], F32, tag="avA")
                        avB = av_ps.tile([65, 1024], F32, tag="avB")
                        for j in range(16):  # key blocks
                            klo = j * 128
                            psA = sc_ps.tile([128, 1024], F32, tag="psA")
                            psB = sc_ps.tile([128, 1024], F32, tag="psB")
                            for h in range(2):
                                ps = (psA, psB)[h]
                                for sub in range(2):
                                    nc.tensor.matmul(
                                        ps[:, sub * 512:(sub + 1) * 512],
                                        kT[b][h * 64:(h + 1) * 64,
                                              klo:klo + 128],
                                        qT[b][h * 64:(h + 1) * 64,
                                              qlo + sub * 512:
                                              qlo + (sub + 1) * 512],
                                        start=True, stop=True,
                                        tile_position=(64 * h, 0))
                            pTA = pt_pool.tile([128, 1024], F32R, tag="pTA")
                            pTB = pt_pool.tile([128, 1024], F32R, tag="pTB")
                            nc.scalar.activation(pTA[:], psA[:], EXP,
                                                 scale=SCALE)
                            nc.scalar.activation(pTB[:], psB[:], EXP,
                                                 scale=SCALE)
                            tb = b * 16 + j
                            for h, (av, pT) in enumerate(((avA, pTA),
                                                          (avB, pTB))):
                                for sub in range(2):
                                    nc.tensor.matmul(
                                        av[:, sub * 512:(sub + 1) * 512],
                                        v_aug[tb][:, h, :],
                                        pT[:, sub * 512:(sub + 1) * 512],
                                        start=(j == 0), stop=(j == 15))
                        for h, av in enumerate((avA, avB)):
                            rec = rc_pool.tile([1, 1024], F32, tag="rec")
                            nc.vector.reciprocal(rec[:], av[64:65, :])
                            bc = rc_pool.tile([64, 1024], F32, tag="bc")
                            nc.gpsimd.partition_broadcast(bc[:], rec[:])
                            nc.vector.tensor_mul(
                                aT[b][h * 64:(h + 1) * 64, qlo:qlo + 1024],
                                av[0:64, :], bc[:])

            # ---- exchange + output projection ---------------------------
            a2a_in = dram.tile([NCORES, 128, 512], F32R, name="a2a_in")
            a2a_out = dram.tile([NCORES, 128, 512], F32R, name="a2a_out")
            for p in range(NCORES):
                b, ch = p // 4, p % 4
                nc.sync.dma_start(a2a_in[p],
                                  aT[b][:, ch * 512:(ch + 1) * 512])
            nc.gpsimd.collective_compute(
                "AllToAll", mybir.AluOpType.bypass,
                ins=[a2a_in[:]], outs=[a2a_out[:]], replica_groups=groups)

            with tc.tile_pool(name="ko", bufs=KB) as ko_pool, \
                 tc.tile_pool(name="osb", bufs=2) as osb_pool, \
                 tc.tile_pool(name="o_ps", bufs=2, space="PSUM") as ops:
                ko = []
                for r in range(NCORES):
                    t = ko_pool.tile([128, 512], F32R, tag="ko", name=f"ko{r}")
                    nc.sync.dma_start(t[:], a2a_out[r])
                    ko.append(t)
                for tb in range(4):
                    for n in range(2):
                        acc = ops.tile([128, 512], F32, tag="oacc")
                        for r in range(NCORES):
                            nc.tensor.matmul(
                                acc[:], ko[r][:, tb * 128:(tb + 1) * 128],
                                wo_sb[r][:, n * 512:(n + 1) * 512],
                                start=(r == 0), stop=(r == NCORES - 1))
                        ot = osb_pool.tile([128, 512], F32, tag="ot")
                        nc.vector.tensor_add(ot[:], acc[:],
                                             bo_bc[:, n * 512:(n + 1) * 512])
                        nc.sync.dma_start(
                            out[tb * 128:(tb + 1) * 128,
                                n * 512:(n + 1) * 512], ot[:])

    nc.compile()
    return nc


def _get_nc():
    if "nc" not in _CACHE:
        _CACHE["nc"] = _build()
    return _CACHE["nc"]


def _make_in_maps(hidden_states, Wq, bq, Wk, bk, Wv, bv, Wo, bo):
    hs = np.ascontiguousarray(np.asarray(hidden_states, dtype=np.float32))
    xT = np.ascontiguousarray(
        hs.reshape(BT, D).T)  # [D, 4096], col = b*2048+t
    eye = np.eye(128, dtype=np.float32)
    vones = np.ones((128, 130), dtype=np.float32)
    Wq = np.asarray(Wq, np.float32); Wk = np.asarray(Wk, np.float32)
    Wv = np.asarray(Wv, np.float32); Wo = np.asarray(Wo, np.float32)
    bq = np.asarray(bq, np.float32); bk = np.asarray(bk, np.float32)
    bv = np.asarray(bv, np.float32); bo = np.asarray(bo, np.float32)
    in_maps = []
    for c in range(NCORES):
        sl = slice(2 * c * HD, (2 * c + 2) * HD)  # this core's 2 heads
        in_maps.append({
            "xT": xT,
            "wq": np.ascontiguousarray(Wq[:, sl]),
            "wk": np.ascontiguousarray(Wk[:, sl]),
            "wv": np.ascontiguousarray(Wv[:, sl]),
            "bq": np.ascontiguousarray(bq[sl].reshape(128, 1)),
            "bk": np.ascontiguousarray(bk[sl].reshape(128, 1)),
            "bv": np.ascontiguousarray(bv[sl].reshape(128, 1)),
            "wo": Wo,
            "bo": np.ascontiguousarray(bo.reshape(1, D)),
            "eye": eye,
            "vones": vones,
        })
    return in_maps


def run(trace=False, tmpdir=None, **inputs):
    from concourse.bass_utils import run_bass_kernel_spmd
    nc = _get_nc()
    in_maps = _make_in_maps(**inputs)
    res = run_bass_kernel_spmd(nc, in_maps, list(range(NCORES)), trace=trace,
                               tmpdir=tmpdir)
    full = np.empty((B, S, D), dtype=np.float32)
    for c in range(NCORES):
        b, ch = c // 4, c % 4
        full[b, ch * 512:(ch + 1) * 512, :] = res.results[c]["out"]
    return full, res


def kernel(**inputs) -> np.ndarray:
    out, _ = run(trace=False, **inputs)
    return out


# revision 4
# speedup vs baseline: 1.0207x; 1.0207x over previous
"""Multi-head attention (B=2, S=2048, D=1024, H=16) on 8 Trainium2 NeuronCores.

Sharding: head-parallel attention (2 heads/core, both batches), then an
8-core AllToAll redistributes per-head attention outputs so each core
computes the output projection for one (batch, 512-row) chunk.

Dtypes: projections + attention matmuls run in bf16 (fp32 PSUM
accumulation, fp32 softmax); the normalized attention output and the
final Wo projection run in float32r (TF32-like, ~2^-12 rounding).

Per-core pipeline (core c, heads hA=2c, hB=2c+1):
  - Q^T/K^T: [128(2 heads x 64), 4096(b*2048+t)]; lhsT=W slice, rhs=X^T;
    per-partition bias added by DVE on the PSUM->SBUF copy.
  - V computed transposed (V^T) then PE-transposed into V_aug tiles
    [128 keys, 2 heads, 65]; column 64 stays 1.0 so the AV matmul also
    accumulates softmax denominators.
  - scores^T (keys on partitions): two K=64 matmuls packed as 64x128 row
    tiles (head B via tile_position=(64,0)); exp with fused 1/8 scale on
    ScalarE, [128,1024] PSUM->SBUF; AV accumulates [65, 1024] over 16
    key blocks (denominator in row 64).
  - normalize: DVE reciprocal + GPSIMD partition-broadcast + DVE mul.
  - AllToAll (8 shards [128, 512]) -> A^T_full [1024, 512]; Wo matmul
    + bias -> out [512, 1024].
"""

import numpy as np

B, S, D, H, HD = 2, 2048, 1024, 16, 64
NCORES = 8
BT = B * S
SCALE = 1.0 / 8.0

_CACHE = {}


def _build():
    import concourse.bacc as bacc
    import concourse.tile as tile
    import concourse.mybir as mybir

    F32R = mybir.dt.float32r
    F32 = mybir.dt.float32
    BF16 = mybir.dt.bfloat16
    EXP = mybir.ActivationFunctionType.Exp

    nc = bacc.Bacc("TRN2", target_bir_lowering=False, debug=False,
                   num_devices=NCORES)

    # ---- I/O ------------------------------------------------------------
    xT = nc.dram_tensor("xT", [D, BT], BF16, kind="ExternalInput")
    wq = nc.dram_tensor("wq", [D, 128], BF16, kind="ExternalInput")
    wk = nc.dram_tensor("wk", [D, 128], BF16, kind="ExternalInput")
    wv = nc.dram_tensor("wv", [D, 128], BF16, kind="ExternalInput")
    bq = nc.dram_tensor("bq", [128, 1], F32, kind="ExternalInput")
    bk = nc.dram_tensor("bk", [128, 1], F32, kind="ExternalInput")
    bv = nc.dram_tensor("bv", [128, 1], F32, kind="ExternalInput")
    wo = nc.dram_tensor("wo", [D, D], F32R, kind="ExternalInput")
    bo = nc.dram_tensor("bo", [1, D], F32, kind="ExternalInput")
    eye = nc.dram_tensor("eye", [128, 128], BF16, kind="ExternalInput")
    vones = nc.dram_tensor("vones", [128, 130], BF16, kind="ExternalInput")
    out = nc.dram_tensor("out", [512, D], F32, kind="ExternalOutput")

    NQ = 4            # xT quarters (1024 tokens each)
    KB = D // 128     # 8 contraction blocks
    groups = [list(range(NCORES))]

    with tile.TileContext(nc) as tc:
        from contextlib import ExitStack
        with ExitStack() as ctx:
            persist = ctx.enter_context(tc.tile_pool(name="persist", bufs=1))
            dram = ctx.enter_context(
                tc.tile_pool(name="dram", bufs=1, space="DRAM"))

            # ---- small constant loads (fast, front of DMA queue) --------
            wqkv_sb = []
            for k in range(KB):
                t = persist.tile([128, 384], BF16, tag=f"wqkv{k}",
                                 name=f"wqkv{k}")
                nc.sync.dma_start(t[:, 0:128], wq[k * 128:(k + 1) * 128, :])
                nc.sync.dma_start(t[:, 128:256], wk[k * 128:(k + 1) * 128, :])
                nc.sync.dma_start(t[:, 256:384], wv[k * 128:(k + 1) * 128, :])
                wqkv_sb.append(t)
            bq_sb = persist.tile([128, 1], F32, tag="bq")
            bk_sb = persist.tile([128, 1], F32, tag="bk")
            bv_sb = persist.tile([128, 1], F32, tag="bv")
            nc.sync.dma_start(bq_sb[:], bq[:])
            nc.sync.dma_start(bk_sb[:], bk[:])
            nc.sync.dma_start(bv_sb[:], bv[:])
            eye_sb = persist.tile([128, 128], BF16, tag="eye")
            nc.sync.dma_start(eye_sb[:], eye[:])

            # persistent activations
            qT = [persist.tile([128, S], BF16, tag=f"qT{b}", name=f"qT{b}")
                  for b in range(B)]
            kT = [persist.tile([128, S], BF16, tag=f"kT{b}", name=f"kT{b}")
                  for b in range(B)]
            aT = [persist.tile([128, S], F32R, tag=f"aT{b}", name=f"aT{b}")
                  for b in range(B)]
            v_aug = [persist.tile([128, 2, 65], BF16, tag=f"vaug{tb}",
                                  name=f"vaug{tb}")
                     for tb in range(BT // 128)]

            # ---- projections --------------------------------------------
            with tc.tile_pool(name="xq", bufs=2 * KB + 2) as xq_pool, \
                 tc.tile_pool(name="vtmp", bufs=2) as vtmp_pool, \
                 tc.tile_pool(name="proj_ps", bufs=3, space="PSUM") as pps, \
                 tc.tile_pool(name="vt_ps", bufs=2, space="PSUM") as vps:
                for q in range(NQ):
                    b = q // 2
                    xk = []
                    for k in range(KB):
                        t = xq_pool.tile([128, 1024], BF16, tag="xq",
                                         name=f"xq{q}_{k}")
                        nc.sync.dma_start(
                            t[:], xT[k * 128:(k + 1) * 128,
                                     q * 1024:(q + 1) * 1024])
                        xk.append(t)
                    # ones-init for this quarter's V_aug tiles
                    for tb in range(q * 8, (q + 1) * 8):
                        nc.sync.dma_start(
                            v_aug[tb][:].rearrange("p h d -> p (h d)"),
                            vones[:])
                    lo = (q % 2) * 1024
                    for ch in range(2):
                        cs, ce = ch * 512, (ch + 1) * 512
                        for (wcol, bias, dst) in ((0, bq_sb, qT[b]),
                                                  (128, bk_sb, kT[b])):
                            acc = pps.tile([128, 512], F32, tag="pacc")
                            for k in range(KB):
                                nc.tensor.matmul(
                                    acc[:],
                                    wqkv_sb[k][:, wcol:wcol + 128],
                                    xk[k][:, cs:ce],
                                    start=(k == 0), stop=(k == KB - 1))
                            nc.vector.tensor_scalar_add(
                                dst[:, lo + cs:lo + ce], acc[:], bias[:])
                        acc = pps.tile([128, 512], F32, tag="pacc")
                        for k in range(KB):
                            nc.tensor.matmul(
                                acc[:], wqkv_sb[k][:, 256:384],
                                xk[k][:, cs:ce],
                                start=(k == 0), stop=(k == KB - 1))
                        vt = vtmp_pool.tile([128, 512], BF16, tag="vt")
                        nc.vector.tensor_scalar_add(vt[:], acc[:], bv_sb[:])
                        for blk in range(4):
                            tb = q * 8 + ch * 4 + blk
                            pv = vps.tile([128, 128], BF16, tag="pv")
                            nc.tensor.transpose(
                                pv[:], vt[:, blk * 128:(blk + 1) * 128],
                                eye_sb[:])
                            nc.vector.tensor_copy(
                                v_aug[tb][:, 0:2, 0:64],
                                pv[:].rearrange("p (h d) -> p h d", h=2))

            # wo rhs tiles: emitted after projections so these 4MB don't
            # delay the xq loads; they overlap the attention phase.
            wo_sb = []
            for k in range(KB):
                t = persist.tile([128, D], F32R, tag=f"wo{k}", name=f"wo{k}")
                nc.sync.dma_start(t[:], wo[k * 128:(k + 1) * 128, :])
                wo_sb.append(t)
            bo_row = persist.tile([1, D], F32, tag="bo_row")
            nc.sync.dma_start(bo_row[:], bo[:])
            bo_bc = persist.tile([128, D], F32, tag="bo_bc")
            nc.gpsimd.partition_broadcast(bo_bc[:], bo_row[:])

            # ---- attention ----------------------------------------------
            with tc.tile_pool(name="pt", bufs=2) as pt_pool, \
                 tc.tile_pool(name="rc", bufs=2) as rc_pool, \
                 tc.tile_pool(name="sc_ps", bufs=1, space="PSUM") as sc_ps, \
                 tc.tile_pool(name="av_ps", bufs=1, space="PSUM") as av_ps:
                for b in range(B):
                    for ich in range(2):
                        qlo = ich * 1024
                        avA = av_ps.tile([65, 1024], F32, tag="avA")
                        avB = av_ps.tile([65, 1024], F32, tag="avB")
                        prev = None  # (pTA, pTB) of previous j
                        for j in range(16):
                            klo = j * 128
                            psA = sc_ps.tile([128, 1024], F32, tag="psA")
                            psB = sc_ps.tile([128, 1024], F32, tag="psB")
                            for h in range(2):
                                ps = (psA, psB)[h]
                                for sub in range(2):
                                    nc.tensor.matmul(
                                        ps[:, sub * 512:(sub + 1) * 512],
                                        kT[b][h * 64:(h + 1) * 64,
                                              klo:klo + 128],
                                        qT[b][h * 64:(h + 1) * 64,
                                              qlo + sub * 512:
                                              qlo + (sub + 1) * 512],
                                        start=True, stop=True,
                                        tile_position=(64 * h, 0))
                            # AV of previous j: keeps PE busy while ACT
                            # works on this j's exp, without touching the
                            # single-buffered score banks.
                            if prev is not None:
                                tbp = b * 16 + (j - 1)
                                for h, av in enumerate((avA, avB)):
                                    for sub in range(2):
                                        nc.tensor.matmul(
                                            av[:, sub * 512:(sub + 1) * 512],
                                            v_aug[tbp][:, h, :],
                                            prev[h][:, sub * 512:
                                                    (sub + 1) * 512],
                                            start=(j - 1 == 0), stop=False)
                            pTA = pt_pool.tile([128, 1024], BF16, tag="pTA")
                            pTB = pt_pool.tile([128, 1024], BF16, tag="pTB")
                            nc.scalar.activation(pTA[:], psA[:], EXP,
                                                 scale=SCALE)
                            nc.scalar.activation(pTB[:], psB[:], EXP,
                                                 scale=SCALE)
                            prev = (pTA, pTB)
                        tbp = b * 16 + 15
                        for h, av in enumerate((avA, avB)):
                            for sub in range(2):
                                nc.tensor.matmul(
                                    av[:, sub * 512:(sub + 1) * 512],
                                    v_aug[tbp][:, h, :],
                                    prev[h][:, sub * 512:(sub + 1) * 512],
                                    start=False, stop=True)
                        for h, av in enumerate((avA, avB)):
                            rec = rc_pool.tile([1, 1024], F32, tag="rec")
                            nc.vector.reciprocal(rec[:], av[64:65, :])
                            bc = rc_pool.tile([64, 1024], F32, tag="bc")
                            nc.gpsimd.partition_broadcast(bc[:], rec[:])
                            nc.vector.tensor_mul(
                                aT[b][h * 64:(h + 1) * 64, qlo:qlo + 1024],
                                av[0:64, :], bc[:])

            # ---- exchange + output projection ---------------------------
            a2a_in = dram.tile([NCORES, 128, 512], F32R, name="a2a_in")
            a2a_out = dram.tile([NCORES, 128, 512], F32R, name="a2a_out")
            for p in range(NCORES):
                b, chk = p // 4, p % 4
                nc.sync.dma_start(a2a_in[p],
                                  aT[b][:, chk * 512:(chk + 1) * 512])
            nc.gpsimd.collective_compute(
                "AllToAll", mybir.AluOpType.bypass,
                ins=[a2a_in[:]], outs=[a2a_out[:]], replica_groups=groups)

            with tc.tile_pool(name="ko", bufs=KB) as ko_pool, \
                 tc.tile_pool(name="osb", bufs=2) as osb_pool, \
                 tc.tile_pool(name="o_ps", bufs=2, space="PSUM") as ops:
                ko = []
                for r in range(NCORES):
                    t = ko_pool.tile([128, 512], F32R, tag="ko",
                                     name=f"ko{r}")
                    nc.sync.dma_start(t[:], a2a_out[r])
                    ko.append(t)
                for tb in range(4):
                    for n in range(2):
                        acc = ops.tile([128, 512], F32, tag="oacc")
                        for r in range(NCORES):
                            nc.tensor.matmul(
                                acc[:], ko[r][:, tb * 128:(tb + 1) * 128],
                                wo_sb[r][:, n * 512:(n + 1) * 512],
                                start=(r == 0), stop=(r == NCORES - 1))
                        ot = osb_pool.tile([128, 512], F32, tag="ot")
                        nc.vector.tensor_add(ot[:], acc[:],
                                             bo_bc[:, n * 512:(n + 1) * 512])
                        nc.sync.dma_start(
                            out[tb * 128:(tb + 1) * 128,
                                n * 512:(n + 1) * 512], ot[:])

    nc.compile()
    return nc


def _get_nc():
    if "nc" not in _CACHE:
        _CACHE["nc"] = _build()
    return _CACHE["nc"]


def _make_in_maps(hidden_states, Wq, bq, Wk, bk, Wv, bv, Wo, bo):
    import ml_dtypes
    bf16 = ml_dtypes.bfloat16
    hs = np.ascontiguousarray(np.asarray(hidden_states, dtype=np.float32))
    xT = np.ascontiguousarray(hs.reshape(BT, D).T.astype(bf16))
    eye = np.eye(128, dtype=bf16)
    vones = np.ones((128, 130), dtype=bf16)
    Wq = np.asarray(Wq, np.float32).astype(bf16)
    Wk = np.asarray(Wk, np.float32).astype(bf16)
    Wv = np.asarray(Wv, np.float32).astype(bf16)
    Wo = np.ascontiguousarray(np.asarray(Wo, np.float32))
    bq = np.asarray(bq, np.float32); bk = np.asarray(bk, np.float32)
    bv = np.asarray(bv, np.float32); bo = np.asarray(bo, np.float32)
    in_maps = []
    for c in range(NCORES):
        sl = slice(2 * c * HD, (2 * c + 2) * HD)
        in_maps.append({
            "xT": xT,
            "wq": np.ascontiguousarray(Wq[:, sl]),
            "wk": np.ascontiguousarray(Wk[:, sl]),
            "wv": np.ascontiguousarray(Wv[:, sl]),
            "bq": np.ascontiguousarray(bq[sl].reshape(128, 1)),
            "bk": np.ascontiguousarray(bk[sl].reshape(128, 1)),
            "bv": np.ascontiguousarray(bv[sl].reshape(128, 1)),
            "wo": Wo,
            "bo": np.ascontiguousarray(bo.reshape(1, D)),
            "eye": eye,
            "vones": vones,
        })
    return in_maps


def run(trace=False, tmpdir=None, **inputs):
    from concourse.bass_utils import run_bass_kernel_spmd
    nc = _get_nc()
    in_maps = _make_in_maps(**inputs)
    res = run_bass_kernel_spmd(nc, in_maps, list(range(NCORES)), trace=trace,
                               tmpdir=tmpdir)
    full = np.empty((B, S, D), dtype=np.float32)
    for c in range(NCORES):
        b, chk = c // 4, c % 4
        full[b, chk * 512:(chk + 1) * 512, :] = res.results[c]["out"]
    return full, res


def kernel(**inputs) -> np.ndarray:
    out, _ = run(trace=False, **inputs)
    return out


# revision 5
# speedup vs baseline: 1.2600x; 1.2345x over previous
"""Multi-head attention (B=2, S=2048, D=1024, H=16) on 8 Trainium2 NeuronCores.

Sharding: head-parallel attention (2 heads/core, both batches), then an
8-core AllToAll redistributes per-head attention outputs so each core
computes the output projection for one (batch, 512-row) chunk.

Dtypes: projections + attention matmuls run in bf16 (fp32 PSUM
accumulation, fp32 softmax); the normalized attention output and the
final Wo projection run in float32r (TF32-like, ~2^-12 rounding).

Per-core pipeline (core c, heads hA=2c, hB=2c+1):
  - Q^T/K^T: [128(2 heads x 64), 4096(b*2048+t)]; lhsT=W slice, rhs=X^T;
    per-partition bias added by DVE on the PSUM->SBUF copy.
  - V computed transposed (V^T) then PE-transposed into V_aug tiles
    [128 keys, 2 heads, 65]; column 64 stays 1.0 so the AV matmul also
    accumulates softmax denominators.
  - scores^T (keys on partitions): two K=64 matmuls packed as 64x128 row
    tiles (head B via tile_position=(64,0)); exp with fused 1/8 scale on
    ScalarE, [128,1024] PSUM->SBUF; AV accumulates [65, 1024] over 16
    key blocks (denominator in row 64).
  - normalize: DVE reciprocal + GPSIMD partition-broadcast + DVE mul.
  - AllToAll (8 shards [128, 512]) -> A^T_full [1024, 512]; Wo matmul
    + bias -> out [512, 1024].
"""

import numpy as np

B, S, D, H, HD = 2, 2048, 1024, 16, 64
NCORES = 8
BT = B * S
SCALE = 1.0 / 8.0

_CACHE = {}


def _build():
    import concourse.bacc as bacc
    import concourse.tile as tile
    import concourse.mybir as mybir

    F32R = mybir.dt.float32r
    F32 = mybir.dt.float32
    BF16 = mybir.dt.bfloat16
    EXP = mybir.ActivationFunctionType.Exp

    nc = bacc.Bacc("TRN2", target_bir_lowering=False, debug=False,
                   num_devices=NCORES)

    # ---- I/O ------------------------------------------------------------
    xT = nc.dram_tensor("xT", [D, BT], BF16, kind="ExternalInput")
    wq = nc.dram_tensor("wq", [D, 128], BF16, kind="ExternalInput")
    wk = nc.dram_tensor("wk", [D, 128], BF16, kind="ExternalInput")
    wv = nc.dram_tensor("wv", [D, 128], BF16, kind="ExternalInput")
    bq = nc.dram_tensor("bq", [128, 1], F32, kind="ExternalInput")
    bk = nc.dram_tensor("bk", [128, 1], F32, kind="ExternalInput")
    bv = nc.dram_tensor("bv", [128, 1], F32, kind="ExternalInput")
    wo = nc.dram_tensor("wo", [D, D], F32R, kind="ExternalInput")
    bo = nc.dram_tensor("bo", [1, D], F32, kind="ExternalInput")
    eye = nc.dram_tensor("eye", [128, 128], BF16, kind="ExternalInput")
    vones = nc.dram_tensor("vones", [128, 130], BF16, kind="ExternalInput")
    out = nc.dram_tensor("out", [512, D], F32, kind="ExternalOutput")

    NQ = 4            # xT quarters (1024 tokens each)
    KB = D // 128     # 8 contraction blocks
    groups = [list(range(NCORES))]

    with tile.TileContext(nc) as tc:
        from contextlib import ExitStack
        with ExitStack() as ctx:
            persist = ctx.enter_context(tc.tile_pool(name="persist", bufs=1))
            dram = ctx.enter_context(
                tc.tile_pool(name="dram", bufs=1, space="DRAM"))

            # ---- small constant loads (fast, front of DMA queue) --------
            wqkv_sb = []
            for k in range(KB):
                t = persist.tile([128, 384], BF16, tag=f"wqkv{k}",
                                 name=f"wqkv{k}")
                nc.sync.dma_start(t[:, 0:128], wq[k * 128:(k + 1) * 128, :])
                nc.sync.dma_start(t[:, 128:256], wk[k * 128:(k + 1) * 128, :])
                nc.sync.dma_start(t[:, 256:384], wv[k * 128:(k + 1) * 128, :])
                wqkv_sb.append(t)
            bq_sb = persist.tile([128, 1], F32, tag="bq")
            bk_sb = persist.tile([128, 1], F32, tag="bk")
            bv_sb = persist.tile([128, 1], F32, tag="bv")
            nc.sync.dma_start(bq_sb[:], bq[:])
            nc.sync.dma_start(bk_sb[:], bk[:])
            nc.sync.dma_start(bv_sb[:], bv[:])
            eye_sb = persist.tile([128, 128], BF16, tag="eye")
            nc.sync.dma_start(eye_sb[:], eye[:])

            # persistent activations
            qT = [persist.tile([128, S], BF16, tag=f"qT{b}", name=f"qT{b}")
                  for b in range(B)]
            kT = [persist.tile([128, S], BF16, tag=f"kT{b}", name=f"kT{b}")
                  for b in range(B)]
            aT = [persist.tile([128, S], F32R, tag=f"aT{b}", name=f"aT{b}")
                  for b in range(B)]
            v_aug = [persist.tile([128, 2, 65], BF16, tag=f"vaug{tb}",
                                  name=f"vaug{tb}")
                     for tb in range(BT // 128)]

            # ---- projections --------------------------------------------
            with tc.tile_pool(name="xq", bufs=2 * KB + 2) as xq_pool, \
                 tc.tile_pool(name="vtmp", bufs=2) as vtmp_pool, \
                 tc.tile_pool(name="proj_ps", bufs=3, space="PSUM") as pps, \
                 tc.tile_pool(name="vt_ps", bufs=2, space="PSUM") as vps:
                for q in range(NQ):
                    b = q // 2
                    xk = []
                    for k in range(KB):
                        t = xq_pool.tile([128, 1024], BF16, tag="xq",
                                         name=f"xq{q}_{k}")
                        nc.sync.dma_start(
                            t[:], xT[k * 128:(k + 1) * 128,
                                     q * 1024:(q + 1) * 1024])
                        xk.append(t)
                    # ones-init for this quarter's V_aug tiles
                    for tb in range(q * 8, (q + 1) * 8):
                        nc.sync.dma_start(
                            v_aug[tb][:].rearrange("p h d -> p (h d)"),
                            vones[:])
                    lo = (q % 2) * 1024
                    vts = []
                    for ch in range(2):
                        cs, ce = ch * 512, (ch + 1) * 512
                        for (wcol, bias, dst) in ((0, bq_sb, qT[b]),
                                                  (128, bk_sb, kT[b])):
                            acc = pps.tile([128, 512], F32, tag="pacc")
                            for k in range(KB):
                                nc.tensor.matmul(
                                    acc[:],
                                    wqkv_sb[k][:, wcol:wcol + 128],
                                    xk[k][:, cs:ce],
                                    start=(k == 0), stop=(k == KB - 1))
                            nc.vector.tensor_scalar_add(
                                dst[:, lo + cs:lo + ce], acc[:], bias[:])
                        acc = pps.tile([128, 512], F32, tag="pacc")
                        for k in range(KB):
                            nc.tensor.matmul(
                                acc[:], wqkv_sb[k][:, 256:384],
                                xk[k][:, cs:ce],
                                start=(k == 0), stop=(k == KB - 1))
                        vt = vtmp_pool.tile([128, 512], BF16, tag="vt")
                        nc.vector.tensor_scalar_add(vt[:], acc[:], bv_sb[:])
                        vts.append(vt)
                    # transposes grouped: one transpose-mode region/quarter
                    for ch, vt in enumerate(vts):
                        for blk in range(4):
                            tb = q * 8 + ch * 4 + blk
                            pv = vps.tile([128, 128], BF16, tag="pv")
                            nc.tensor.transpose(
                                pv[:], vt[:, blk * 128:(blk + 1) * 128],
                                eye_sb[:])
                            nc.vector.tensor_copy(
                                v_aug[tb][:, 0:2, 0:64],
                                pv[:].rearrange("p (h d) -> p h d", h=2))

            # wo rhs tiles: emitted after projections so these 4MB don't
            # delay the xq loads; they overlap the attention phase.
            wo_sb = []
            for k in range(KB):
                t = persist.tile([128, D], F32R, tag=f"wo{k}", name=f"wo{k}")
                nc.sync.dma_start(t[:], wo[k * 128:(k + 1) * 128, :])
                wo_sb.append(t)
            bo_row = persist.tile([1, D], F32, tag="bo_row")
            nc.sync.dma_start(bo_row[:], bo[:])
            bo_bc = persist.tile([128, D], F32, tag="bo_bc")
            nc.gpsimd.partition_broadcast(bo_bc[:], bo_row[:])

            # ---- attention ----------------------------------------------
            with tc.tile_pool(name="pt", bufs=3) as pt_pool, \
                 tc.tile_pool(name="rc", bufs=2) as rc_pool, \
                 tc.tile_pool(name="sc_ps", bufs=1, space="PSUM") as sc_ps, \
                 tc.tile_pool(name="av_ps", bufs=1, space="PSUM") as av_ps:
                for b in range(B):
                    for ich in range(2):
                        qlo = ich * 1024
                        avA = av_ps.tile([65, 1024], F32, tag="avA")
                        avB = av_ps.tile([65, 1024], F32, tag="avB")
                        prev = None  # (pTA, pTB) of previous j
                        for j in range(16):
                            klo = j * 128
                            psA = sc_ps.tile([128, 1024], F32, tag="psA")
                            psB = sc_ps.tile([128, 1024], F32, tag="psB")
                            for h in range(2):
                                ps = (psA, psB)[h]
                                for sub in range(2):
                                    nc.tensor.matmul(
                                        ps[:, sub * 512:(sub + 1) * 512],
                                        kT[b][h * 64:(h + 1) * 64,
                                              klo:klo + 128],
                                        qT[b][h * 64:(h + 1) * 64,
                                              qlo + sub * 512:
                                              qlo + (sub + 1) * 512],
                                        start=True, stop=True)
                            # AV of previous j: keeps PE busy while ACT
                            # works on this j's exp, without touching the
                            # single-buffered score banks.
                            if prev is not None:
                                tbp = b * 16 + (j - 1)
                                for h, av in enumerate((avA, avB)):
                                    for sub in range(2):
                                        nc.tensor.matmul(
                                            av[:, sub * 512:(sub + 1) * 512],
                                            v_aug[tbp][:, h, :],
                                            prev[h][:, sub * 512:
                                                    (sub + 1) * 512],
                                            start=(j - 1 == 0), stop=False)
                            pTA = pt_pool.tile([128, 1024], BF16, tag="pTA")
                            pTB = pt_pool.tile([128, 1024], BF16, tag="pTB")
                            nc.scalar.activation(pTA[:], psA[:], EXP,
                                                 scale=SCALE)
                            nc.scalar.activation(pTB[:], psB[:], EXP,
                                                 scale=SCALE)
                            prev = (pTA, pTB)
                        tbp = b * 16 + 15
                        for h, av in enumerate((avA, avB)):
                            for sub in range(2):
                                nc.tensor.matmul(
                                    av[:, sub * 512:(sub + 1) * 512],
                                    v_aug[tbp][:, h, :],
                                    prev[h][:, sub * 512:(sub + 1) * 512],
                                    start=False, stop=True)
                        for h, av in enumerate((avA, avB)):
                            num = rc_pool.tile([64, 1024], F32, tag="num")
                            nc.vector.tensor_copy(num[:], av[0:64, :])
                            den = rc_pool.tile([1, 1024], F32, tag="den")
                            nc.vector.tensor_copy(den[:], av[64:65, :])
                            rec = rc_pool.tile([1, 1024], F32, tag="rec")
                            nc.vector.reciprocal_approx_fast(rec[:], den[:])
                            bc = rc_pool.tile([64, 1024], F32, tag="bc")
                            nc.gpsimd.partition_broadcast(bc[:], rec[:])
                            nc.vector.tensor_mul(
                                aT[b][h * 64:(h + 1) * 64, qlo:qlo + 1024],
                                num[:], bc[:])

            # ---- exchange + output projection ---------------------------
            a2a_in = dram.tile([NCORES, 128, 512], F32R, name="a2a_in")
            a2a_out = dram.tile([NCORES, 128, 512], F32R, name="a2a_out")
            for p in range(NCORES):
                b, chk = p // 4, p % 4
                nc.sync.dma_start(a2a_in[p],
                                  aT[b][:, chk * 512:(chk + 1) * 512])
            nc.gpsimd.collective_compute(
                "AllToAll", mybir.AluOpType.bypass,
                ins=[a2a_in[:]], outs=[a2a_out[:]], replica_groups=groups)

            with tc.tile_pool(name="ko", bufs=KB) as ko_pool, \
                 tc.tile_pool(name="osb", bufs=2) as osb_pool, \
                 tc.tile_pool(name="o_ps", bufs=2, space="PSUM") as ops:
                ko = []
                for r in range(NCORES):
                    t = ko_pool.tile([128, 512], F32R, tag="ko",
                                     name=f"ko{r}")
                    nc.sync.dma_start(t[:], a2a_out[r])
                    ko.append(t)
                for tb in range(4):
                    for n in range(2):
                        acc = ops.tile([128, 512], F32, tag="oacc")
                        for r in range(NCORES):
                            nc.tensor.matmul(
                                acc[:], ko[r][:, tb * 128:(tb + 1) * 128],
                                wo_sb[r][:, n * 512:(n + 1) * 512],
                                start=(r == 0), stop=(r == NCORES - 1))
                        ot = osb_pool.tile([128, 512], F32, tag="ot")
                        nc.vector.tensor_add(ot[:], acc[:],
                                             bo_bc[:, n * 512:(n + 1) * 512])
                        nc.sync.dma_start(
                            out[tb * 128:(tb + 1) * 128,
                                n * 512:(n + 1) * 512], ot[:])

    nc.compile()
    return nc


def _get_nc():
    if "nc" not in _CACHE:
        _CACHE["nc"] = _build()
    return _CACHE["nc"]


def _make_in_maps(hidden_states, Wq, bq, Wk, bk, Wv, bv, Wo, bo):
    import ml_dtypes
    bf16 = ml_dtypes.bfloat16
    hs = np.ascontiguousarray(np.asarray(hidden_states, dtype=np.float32))
    xT = np.ascontiguousarray(hs.reshape(BT, D).T.astype(bf16))
    eye = np.eye(128, dtype=bf16)
    vones = np.ones((128, 130), dtype=bf16)
    Wq = np.asarray(Wq, np.float32).astype(bf16)
    Wk = np.asarray(Wk, np.float32).astype(bf16)
    Wv = np.asarray(Wv, np.float32).astype(bf16)
    Wo = np.ascontiguousarray(np.asarray(Wo, np.float32))
    bq = np.asarray(bq, np.float32); bk = np.asarray(bk, np.float32)
    bv = np.asarray(bv, np.float32); bo = np.asarray(bo, np.float32)
    in_maps = []
    for c in range(NCORES):
        sl = slice(2 * c * HD, (2 * c + 2) * HD)
        in_maps.append({
            "xT": xT,
            "wq": np.ascontiguousarray(Wq[:, sl]),
            "wk": np.ascontiguousarray(Wk[:, sl]),
            "wv": np.ascontiguousarray(Wv[:, sl]),
            "bq": np.ascontiguousarray(bq[sl].reshape(128, 1)),
            "bk": np.ascontiguousarray(bk[sl].reshape(128, 1)),
            "bv": np.ascontiguousarray(bv[sl].reshape(128, 1)),
            "wo": Wo,
            "bo": np.ascontiguousarray(bo.reshape(1, D)),
            "eye": eye,
            "vones": vones,
        })
    return in_maps


def run(trace=False, tmpdir=None, **inputs):
    from concourse.bass_utils import run_bass_kernel_spmd
    nc = _get_nc()
    in_maps = _make_in_maps(**inputs)
    res = run_bass_kernel_spmd(nc, in_maps, list(range(NCORES)), trace=trace,
                               tmpdir=tmpdir)
    full = np.empty((B, S, D), dtype=np.float32)
    for c in range(NCORES):
        b, chk = c // 4, c % 4
        full[b, chk * 512:(chk + 1) * 512, :] = res.results[c]["out"]
    return full, res


def kernel(**inputs) -> np.ndarray:
    out, _ = run(trace=False, **inputs)
    return out
